# revision 14
# baseline (speedup 1.0000x reference)
"""BiologicallyInformedAttention TRN2 kernel (8 NeuronCores, axon/PJRT).

Sharding: B*H = 32 (batch, head) pairs over 8 cores -> core c handles batch
c//2, heads (c%2)*4 .. +4 (= 2 head-pairs). Projection weights are
column-sliced per core; x is transposed host-side so every matmul contracts
over the partition dim.

v2: PE-paced software pipeline. Per (pair, q-chunk of 512, key-tile of 128):
  sc[128,1024] psum = scoresT both heads (2 row-packed K=64 f32r matmuls,
    tile_position (0,0)/(64,0), ~258ns/pair measured)
  exp on ACT as ONE [128,1024] instr (scale=1/sqrt(dh) fused; ~720ns measured)
  prior applied POST-exp: et diag block *= exp(prior_weight), one DVE op over
    a [128,2,128] AP (epwi doubled tile)
  AV lags scores by one kt slot so the PE never blocks on ACT
  av[96,512] psum per head += v_aug[128,96]^T @ et over kt; v_aug = [v|1|0pad]
    (row 64 = softmax denominator; cols 65..95 zero-pad N to 96 because the
    f32r fast streaming path needs N a multiple of 32: N=66 measured 234ns,
    N=96 measured 123ns per matmul)
  normalize: av->sbuf copy frees psum; recip + gpsimd partition-broadcast +
    DVE mul are deferred a few slots off the critical path -> attnT[h]
  outproj: 4 accumulating K=64 matmuls -> outT, DMA per 512-chunk
Projections (W.T@xT for q/k; xT.T@Wv + b via K=1 ones-matmul for v) are
threaded one-unit-per-slot as PE filler inside the attention loops; across
repeats the next repeat's projections hide under the current repeat's
attention (steady state for the R-differential timing).
PSUM budget: sc 2bufs x 2banks + av0/av1 1buf x 1bank + pj 2bufs x 1bank = 8.
Host: out[b] = (outT[2b] + outT[2b+1]).T + bo.

Measured (R1-vs-R32 differential, median of 3): 140-146us/pass vs 275.6us for
the previous baseline kernel on the same methodology.
"""
import numpy as np
from contextlib import ExitStack

import concourse.bacc as bacc
import concourse.tile as tile
from concourse import mybir
from concourse.bass_utils import run_bass_kernel_spmd

B, S, D, H, DH = 4, 2048, 512, 8, 64
HPC = H // 2          # heads per core = 4
W_COLS = HPC * DH     # 256 per-core projection columns
N_CORES = 8
NCH = 4               # q-chunks of 512 per pair

f32 = mybir.dt.float32
f32r = mybir.dt.float32r
AF = mybir.ActivationFunctionType

_BUILT = {}


def _build(repeat=1):
    nc = bacc.Bacc("TRN2", target_bir_lowering=False)

    xT_d = nc.declare_dram_parameter("xT", [D, S], f32r, isOutput=False)
    wq_d = nc.declare_dram_parameter("wq", [D, W_COLS], f32r, isOutput=False)
    wk_d = nc.declare_dram_parameter("wk", [D, W_COLS], f32r, isOutput=False)
    wv_d = nc.declare_dram_parameter("wv", [D, W_COLS], f32r, isOutput=False)
    wo_d = nc.declare_dram_parameter("wo", [W_COLS, DH], f32r, isOutput=False)
    bq_d = nc.declare_dram_parameter("bq", [W_COLS, 1], f32, isOutput=False)
    bk_d = nc.declare_dram_parameter("bk", [W_COLS, 1], f32, isOutput=False)
    bv_d = nc.declare_dram_parameter("bv", [1, W_COLS], f32r, isOutput=False)
    epwi_d = nc.declare_dram_parameter("epwi", [128, 256], f32, isOutput=False)
    ones_row_d = nc.declare_dram_parameter("ones_row", [1, 128], f32r, isOutput=False)
    ones_blk_d = nc.declare_dram_parameter("ones_blk", [128, 8], f32r, isOutput=False)
    va_init_d = nc.declare_dram_parameter("va_init", [128, HPC * 96], f32r, isOutput=False)
    outT_d = nc.declare_dram_parameter("outT", [DH, S], f32, isOutput=True)

    with tile.TileContext(nc) as tc, ExitStack() as ctx:
        cp = ctx.enter_context(tc.tile_pool(name="cp", bufs=1))

        # ---------- persistent tiles ----------
        xr = [cp.tile([128, S], f32r, tag=f"xr{i}", name=f"xr{i}") for i in range(4)]
        wqr = [cp.tile([128, W_COLS], f32r, tag=f"wqr{i}", name=f"wqr{i}") for i in range(4)]
        wkr = [cp.tile([128, W_COLS], f32r, tag=f"wkr{i}", name=f"wkr{i}") for i in range(4)]
        wvr = [cp.tile([128, W_COLS], f32r, tag=f"wvr{i}", name=f"wvr{i}") for i in range(4)]
        wor = cp.tile([DH, W_COLS], f32r, tag="wor", name="wor")
        bq_t = cp.tile([128, 2], f32, tag="bq", name="bq")
        bk_t = cp.tile([128, 2], f32, tag="bk", name="bk")
        bvr = cp.tile([1, W_COLS], f32r, tag="bvr", name="bvr")
        ones_col = cp.tile([1, 128], f32r, tag="ones_col", name="ones_col")
        ones_blk = cp.tile([128, 8], f32r, tag="ones_blk", name="ones_blk")
        epwi = cp.tile([128, 256], f32, tag="epwi", name="epwi")
        qTr = [cp.tile([128, S], f32r, tag=f"qTr{p}", name=f"qTr{p}") for p in range(2)]
        kTr = [cp.tile([128, S], f32r, tag=f"kTr{p}", name=f"kTr{p}") for p in range(2)]
        v_aug = [cp.tile([128, HPC * 96], f32r, tag=f"va{st}", name=f"va{st}") for st in range(16)]
        attnT = [cp.tile([DH, S], f32r, tag=f"at{h}", name=f"at{h}") for h in range(HPC)]
        outT_s = cp.tile([DH, S], f32, tag="outT", name="outT")

        # ---------- loads (all f32r DMA-direct) ----------
        for di in range(4):
            nc.sync.dma_start(wqr[di][:], wq_d[di * 128:(di + 1) * 128, :])
            nc.sync.dma_start(wkr[di][:], wk_d[di * 128:(di + 1) * 128, :])
        for sc4 in range(4):
            s0 = sc4 * 512
            for di in range(4):
                nc.sync.dma_start(xr[di][:, s0:s0 + 512],
                                  xT_d[di * 128:(di + 1) * 128, s0:s0 + 512])
            if sc4 == 1:
                for di in range(4):
                    nc.sync.dma_start(wvr[di][:], wv_d[di * 128:(di + 1) * 128, :])
        for h in range(HPC):
            nc.sync.dma_start(wor[:, h * DH:(h + 1) * DH],
                              wo_d[h * DH:(h + 1) * DH, :])
        for ht in range(2):
            nc.sync.dma_start(bq_t[:, ht:ht + 1], bq_d[ht * 128:(ht + 1) * 128, :])
            nc.sync.dma_start(bk_t[:, ht:ht + 1], bk_d[ht * 128:(ht + 1) * 128, :])
        nc.sync.dma_start(bvr[:], bv_d[:])
        nc.sync.dma_start(ones_col[:], ones_row_d[:])
        nc.sync.dma_start(ones_blk[:], ones_blk_d[:])
        nc.sync.dma_start(epwi[:], epwi_d[:])
        # ones (col 64) and zero-pad (cols 65..95) of v_aug never change:
        # DMA the pattern once. Cols 65..95 pad N to 96 so the AV matmul hits
        # the fast f32r streaming path (N=66 runs ~1.7x slower).
        for st in range(16):
            nc.sync.dma_start(v_aug[st][:], va_init_d[:])

        with tc.tile_pool(name="scp", bufs=2, space="PSUM") as scp, \
             tc.tile_pool(name="avp", bufs=1, space="PSUM") as avp, \
             tc.tile_pool(name="pjp", bufs=2, space="PSUM") as pjp, \
             tc.tile_pool(name="etp", bufs=6) as etp, \
             tc.tile_pool(name="nrm", bufs=2) as nrm:

            # ---- projection units (PE filler) ----
            def qk_unit_thunks(ht, sc4, which):
                """5 thunks: 4 accumulating matmuls + 1 evac (bias add)."""
                wr = wqr if which == "q" else wkr
                bias_t = bq_t if which == "q" else bk_t
                dst = qTr if which == "q" else kTr
                s0 = sc4 * 512
                box = {}

                def mk_mm(di):
                    def t():
                        if di == 0:
                            box["pt"] = pjp.tile([128, 512], f32, tag="pj", name="pj")
                        nc.tensor.matmul(
                            box["pt"][:],
                            wr[di][:, ht * 128:(ht + 1) * 128],
                            xr[di][:, s0:s0 + 512],
                            start=(di == 0), stop=(di == 3))
                    return t

                def evac():
                    nc.vector.tensor_scalar_add(
                        dst[ht][:, s0:s0 + 512], box["pt"][:],
                        bias_t[:, ht:ht + 1])

                return [mk_mm(d) for d in range(4)] + [evac]

            def v_unit(st):
                def t():
                    pv = pjp.tile([128, W_COLS], f32, tag="pj", name="pv")
                    for di in range(4):
                        nc.tensor.matmul(pv[:],
                                         xr[di][:, st * 128:(st + 1) * 128],
                                         wvr[di][:],
                                         start=(di == 0), stop=False)
                    nc.tensor.matmul(pv[:], ones_col[:], bvr[:],
                                     start=False, stop=True)
                    va = v_aug[st][:].rearrange("p (h c) -> p h c", c=96)
                    nc.vector.tensor_copy(
                        va[:, :, 0:DH],
                        pv[:].rearrange("p (h c) -> p h c", c=DH))
                return t

            def outproj_unit(ch):
                def t():
                    s0 = ch * 512
                    po = pjp.tile([DH, 512], f32, tag="pj", name="po")
                    for h in range(HPC):
                        nc.tensor.matmul(po[:],
                                         wor[:, h * DH:(h + 1) * DH],
                                         attnT[h][:, s0:s0 + 512],
                                         start=(h == 0), stop=(h == HPC - 1))
                    nc.vector.tensor_copy(outT_s[:, s0:s0 + 512], po[:])
                    nc.sync.dma_start(outT_d[:, s0:s0 + 512], outT_s[:, s0:s0 + 512])
                return t

            # ---- attention over one head-pair ----
            # state["av_pending"] is a closure doing the AV matmuls of the
            # previous kt slot (plus, on the last kt of a chunk, the chunk's
            # normalize). It is executed after the NEXT slot's score matmuls
            # so the PE never sits waiting on the ACT exp of the current kt.
            state = {"av_pending": [], "deferred": []}

            def flush_av(keep=0):
                while len(state["av_pending"]) > keep:
                    state["av_pending"].pop(0)()

            def normalize(p, ch, av0, av1):
                # Evacuate av psum now (frees the banks for the next chunk);
                # defer the recip/broadcast/mul tail a couple of slots so it
                # does not queue ahead of latency-critical DVE work.
                q0 = ch * 512
                tails = []
                for hh, av in ((0, av0), (1, av1)):
                    avs = nrm.tile([65, 512], f32, tag=f"avs{hh}", name="avs")
                    nc.vector.tensor_copy(avs[:], av[0:65, :])
                    sums = nrm.tile([1, 512], f32, tag=f"sums{hh}", name="sums")
                    nc.vector.tensor_copy(sums[:], avs[DH:DH + 1, :])

                    def tail(hh=hh, avs=avs, sums=sums):
                        rcp = nrm.tile([1, 512], f32, tag=f"rcp{hh}", name="rcp")
                        nc.vector.reciprocal_approx_fast(rcp[:], sums[:])
                        rB = nrm.tile([DH, 512], f32, tag=f"rB{hh}", name="rB")
                        nc.gpsimd.partition_broadcast(rB[:], rcp[:])
                        nc.vector.tensor_mul(
                            attnT[2 * p + hh][:, q0:q0 + 512],
                            avs[0:DH, :], rB[:])
                    tails.append(tail)
                state["deferred"].extend(tails)

            def attention_pair(p, inserts):
                h0, h1 = 2 * p, 2 * p + 1
                for ch in range(NCH):
                    q0 = ch * 512
                    av0 = avp.tile([96, 512], f32, tag="av0", name="av0")
                    av1 = avp.tile([96, 512], f32, tag="av1", name="av1")
                    for kt in range(16):
                        k0 = kt * 128
                        sc = scp.tile([128, 1024], f32, tag="sc", name="sc")
                        for et_half, base in ((0, 0), (512, 64)):
                            nc.tensor.matmul(
                                sc[:, et_half:et_half + 512],
                                kTr[p][base:base + 64, k0:k0 + 128],
                                qTr[p][base:base + 64, q0:q0 + 512],
                                start=True, stop=True,
                                tile_position=(base, 0))
                        et = etp.tile([128, 1024], f32r, tag="et", name="et")
                        if not os.environ.get("KNOEXP"):
                            nc.scalar.activation(et[:], sc[:], AF.Exp, scale=0.125)
                        off = k0 - q0
                        if (0 <= off < 512) and not os.environ.get("KNOEXP"):
                            # one DVE op fixes the diag block of both heads
                            et3 = et[:].rearrange("p (t c) -> p t c", c=512)
                            ep3 = epwi[:].rearrange("p (t c) -> p t c", c=128)
                            nc.vector.tensor_mul(et3[:, :, off:off + 128],
                                                 et3[:, :, off:off + 128], ep3)
                        # AV lagged 2 slots: its et sem is long satisfied
                        flush_av(keep=2)
                        if state["deferred"]:
                            state["deferred"].pop(0)()
                        if inserts:
                            th = inserts.pop(0)
                            if th is not None:
                                th()

                        def mk_av(et=et, av0=av0, av1=av1, kt=kt, p=p, ch=ch,
                                  h0=h0, h1=h1):
                            def t():
                                if not os.environ.get("KNOAV"):
                                    nc.tensor.matmul(
                                        av0[:], v_aug[kt][:, h0 * 96:h0 * 96 + 96],
                                        et[:, 0:512],
                                        start=(kt == 0), stop=(kt == 15))
                                    nc.tensor.matmul(
                                        av1[:], v_aug[kt][:, h1 * 96:h1 * 96 + 96],
                                        et[:, 512:1024],
                                        start=(kt == 0), stop=(kt == 15))
                                elif kt == 15:
                                    nc.tensor.matmul(
                                        av0[:], v_aug[kt][:, h0 * 96:h0 * 96 + 96],
                                        et[:, 0:512], start=True, stop=True)
                                    nc.tensor.matmul(
                                        av1[:], v_aug[kt][:, h1 * 96:h1 * 96 + 96],
                                        et[:, 512:1024], start=True, stop=True)
                                if kt == 15:
                                    normalize(p, ch, av0, av1)
                            return t
                        state["av_pending"].append(mk_av())

            # ---- repeat loop with software-pipelined projections ----
            def qk_all(ht):
                out = []
                for sc4 in range(4):
                    for w in ("q", "k"):
                        out += qk_unit_thunks(ht, sc4, w)
                return out

            if os.environ.get("KNOPIPE"):
                for r in range(repeat):
                    for th in qk_all(0) + qk_all(1):
                        th()
                    for st in range(16):
                        v_unit(st)()
                    attention_pair(0, [])
                    attention_pair(1, [])
                    flush_av()
                    for ch in range(NCH):
                        outproj_unit(ch)()
            else:
                pending = []
                for r in range(repeat):
                    last = (r == repeat - 1)
                    # -- pair-0 inserts: leftovers from prev pair-1 + qk1(r) --
                    if r == 0:
                        for th in qk_all(0):
                            th()
                        v_unit(0)()
                        v_unit(1)()
                        ins0 = [v_unit(st) for st in range(2, 16)]
                    else:
                        ins0 = list(pending)
                        pending = []
                    ins0 += qk_all(1)
                    attention_pair(0, ins0)

                    # ---- pair-1 inserts (64 slots) ----
                    ins1 = [None] * 64
                    slots = [s for s in range(64) if s not in (20, 36)]
                    fill = qk_all(0) if not last else []
                    for i, th in enumerate(fill):
                        ins1[slots[i]] = th
                    ins1[20] = outproj_unit(0)
                    ins1[36] = outproj_unit(1)
                    if not last:
                        # v(st) for next repeat once chunk-3's AV(st) has read
                        # the old tile: AV(st) runs at slot 48+st+1 -> 50+st.
                        for st in range(14):
                            ins1[50 + st] = v_unit(st)
                        pending = [outproj_unit(2), v_unit(14), v_unit(15),
                                   None, None, None, outproj_unit(3)]
                    else:
                        ins1[52] = outproj_unit(2)
                    attention_pair(1, ins1)
                    if last:
                        flush_av()
                        while state["deferred"]:
                            state["deferred"].pop(0)()
                        outproj_unit(3)()
                flush_av()
            while state["deferred"]:
                state["deferred"].pop(0)()

    nc.finalize()
    return nc


def _get_nc(repeat=1):
    if repeat not in _BUILT:
        _BUILT[repeat] = _build(repeat)
    return _BUILT[repeat]


def _make_in_maps(x, Wq, bq, Wk, bk, Wv, bv, Wo, bo, prior_weight):
    pw = float(prior_weight[0])
    epwi1 = np.ones((128, 128), np.float32)
    np.fill_diagonal(epwi1, np.exp(np.float32(pw)))
    epwi = np.concatenate([epwi1, epwi1], axis=1)
    ones_row = np.ones((1, 128), np.float32)
    ones_blk = np.ones((128, 8), np.float32)
    va_init = np.zeros((128, HPC * 96), np.float32)
    for h in range(HPC):
        va_init[:, h * 96 + DH] = 1.0
    xT = [np.ascontiguousarray(x[b].T) for b in range(B)]
    in_maps = []
    for c in range(N_CORES):
        b, half = c // 2, c % 2
        cs = slice(half * W_COLS, (half + 1) * W_COLS)
        in_maps.append({
            "xT": xT[b],
            "wq": np.ascontiguousarray(Wq[:, cs]),
            "wk": np.ascontiguousarray(Wk[:, cs]),
            "wv": np.ascontiguousarray(Wv[:, cs]),
            "wo": np.ascontiguousarray(Wo[cs, :]),
            "bq": np.ascontiguousarray(bq[cs].reshape(W_COLS, 1)),
            "bk": np.ascontiguousarray(bk[cs].reshape(W_COLS, 1)),
            "bv": np.ascontiguousarray(bv[cs].reshape(1, W_COLS)),
            "epwi": epwi,
            "ones_row": ones_row,
            "ones_blk": ones_blk,
            "va_init": va_init,
        })
    return in_maps


def run(inputs, trace=False, trace_cores=None):
    """Execute on 8 cores; returns (output [B,S,DH] f32, BassKernelResults)."""
    args = {k: np.asarray(v) for k, v in inputs.items()}
    nc = _get_nc()
    in_maps = _make_in_maps(
        args["x"], args["Wq"], args["bq"], args["Wk"], args["bk"],
        args["Wv"], args["bv"], args["Wo"], args["bo"], args["prior_weight"])
    res = run_bass_kernel_spmd(
        nc, in_maps, list(range(N_CORES)), trace=trace,
        **({"trace_cores": trace_cores} if trace_cores else {}))
    bo = args["bo"].astype(np.float32)
    out = np.empty((B, S, DH), np.float32)
    for b in range(B):
        acc = res.results[2 * b]["outT"] + res.results[2 * b + 1]["outT"]
        out[b] = acc.T + bo
    return out, res


def kernel(**inputs) -> np.ndarray:
    out, _ = run(inputs, trace=False)
    return out
